# revision 6
# baseline (speedup 1.0000x reference)
"""Trainium2 Bass kernel for the FCBlock weight-transform + matmul problem.

Math (per reference):
    W_i = per-head 3x3 conv over W.reshape(4, 1024, 4096) + conv_b
          + sigmoid(sk_wt) * W            (per-head scalars)
    out  = inp @ W_i.T                    (inp: [2, 2048, 4096])

Strategy: tensor-parallel shard of W along fout across 8 NeuronCores
(512 rows each; the host pre-slices W with 1-row conv halo, zero-padded
at head boundaries).  On each core:
  - build the 3x3 conv as tiny banded matrices (from conv_w/conv_b/sk_wt,
    broadcast on device) and run the weight transform as PE band-matmuls
    accumulating in PSUM (sigmoid-gated residual folded into the center
    diagonal; bias added during the PSUM->SBUF copy),
  - transpose W_i on the PE (fin onto partitions),
  - stream inp tiles (DMA f32->bf16 cast), PE-transpose them, and run the
    main matmul in bf16 with fp32 PSUM accumulation.
Output is sharded on fout; the host concatenates.
"""

import numpy as np

import concourse.bass as bass
import concourse.mybir as mybir
import concourse.tile as tile
from concourse import bacc
from concourse.bass_utils import run_bass_kernel_spmd
from concourse.masks import make_identity

F32 = mybir.dt.float32
BF16 = mybir.dt.bfloat16

NCORES = 8
NUM_HEADS = 4
TOK = 4096          # 2 * 2048 tokens
FIN = 4096
FOUT = 4096
FSH = FOUT // NCORES  # 512 fout rows per core


def build_program(tok=TOK, fin=FIN, repeat=1):
    """Build the per-core SPMD program.

    tok/fin are parameters so a mini variant can be compiled quickly for
    validation; the graded path always uses the full sizes.
    """
    assert tok % 128 == 0 and fin % 512 == 0
    n_tblk = tok // 128          # 128-token blocks
    n_strip = fin // 512         # 512-col fin strips
    n_k = fin // 128             # 128-deep contraction blocks
    n_win = FSH // 128           # 4 fout row windows per core

    nc = bacc.Bacc(None, target_bir_lowering=False)

    inp = nc.declare_dram_parameter("inp", [tok, fin], F32, isOutput=False)
    wh = nc.declare_dram_parameter("wh", [FSH + 2, fin + 2], F32, isOutput=False)
    sc = nc.declare_dram_parameter("sc", [1, 11], F32, isOutput=False)
    out = nc.declare_dram_parameter("o", [tok, FSH], F32, isOutput=True)

    with tile.TileContext(nc) as tc:
        with (
            tc.tile_pool(name="const", bufs=1) as const,
            tc.tile_pool(name="wipool", bufs=1) as wipool,
            tc.tile_pool(name="wtpool", bufs=1) as wtpool,
            tc.tile_pool(name="wrow", bufs=2) as wrowp,
            tc.tile_pool(name="xb", bufs=3) as xbp,
            tc.tile_pool(name="xt", bufs=2) as xtp,
            tc.tile_pool(name="osb", bufs=3) as osbp,
            tc.tile_pool(name="psw", bufs=2, space="PSUM") as psw,
            tc.tile_pool(name="pst", bufs=2, space="PSUM") as pst,
            tc.tile_pool(name="psx", bufs=2, space="PSUM") as psx,
            tc.tile_pool(name="pso", bufs=2, space="PSUM") as pso,
        ):
            # ---- setup: scalars, identity, band matrices -------------------
            ident = const.tile([128, 128], BF16)
            make_identity(nc, ident[:])

            sc_sb = const.tile([1, 11], F32)
            nc.sync.dma_start(out=sc_sb[:], in_=sc[:])

            ones_r = const.tile([1, 128], F32)
            nc.vector.memset(ones_r[:], 1.0)

            # broadcast the 11 scalars to all 128 partitions via a k=1 matmul
            ps_b = psw.tile([128, 11], F32, tag="pw")
            nc.tensor.matmul(ps_b[:], ones_r[:], sc_sb[:], start=True, stop=True)
            scv = const.tile([128, 11], F32)
            nc.vector.tensor_copy(out=scv[:], in_=ps_b[:])

            # ctr = conv_w[h,1,1] + sigmoid(sk_wt[h])
            sig = const.tile([128, 1], F32)
            nc.scalar.activation(sig[:], scv[:, 10:11],
                                 mybir.ActivationFunctionType.Sigmoid)
            ctr = const.tile([128, 1], F32)
            nc.vector.tensor_tensor(out=ctr[:], in0=sig[:], in1=scv[:, 4:5],
                                    op=mybir.AluOpType.add)

            # band matrices B_dc[k, o] = cw[h, k-o, dc] (k-o in {0,1,2});
            # the dc=1 center diagonal also carries the sigmoid residual.
            masks = []
            for d in range(3):
                m = const.tile([128, 128], F32, tag=f"mask{d}")
                nc.gpsimd.memset(m[:], 0.0)
                nc.gpsimd.affine_select(
                    out=m[:], in_=m[:],
                    compare_op=mybir.AluOpType.not_equal,
                    fill=1.0, base=-d, channel_multiplier=1,
                    pattern=[[-1, 128]],
                )
                masks.append(m)
            b_bf = []
            for dc in range(3):
                bf_ = const.tile([128, 128], F32, tag=f"bf_{dc}")
                nc.vector.tensor_scalar(bf_[:], masks[0][:], scv[:, dc:dc + 1],
                                        None, mybir.AluOpType.mult)
                mid = ctr if dc == 1 else scv[:, 3 + dc:4 + dc]
                nc.vector.scalar_tensor_tensor(
                    out=bf_[:], in0=masks[1][:], scalar=mid, in1=bf_[:],
                    op0=mybir.AluOpType.mult, op1=mybir.AluOpType.add)
                nc.vector.scalar_tensor_tensor(
                    out=bf_[:], in0=masks[2][:], scalar=scv[:, 6 + dc:7 + dc],
                    in1=bf_[:],
                    op0=mybir.AluOpType.mult, op1=mybir.AluOpType.add)
                bb = const.tile([128, 128], BF16, tag=f"bb_{dc}")
                nc.vector.tensor_copy(out=bb[:], in_=bf_[:])
                b_bf.append(bb)

            # halo matrices H_dc [2, 128]: out row 127 takes its dr=1/dr=2
            # taps from halo rows 0/1, and out row 126 its dr=2 tap from halo
            # row 0.  Built as outer products (v.T @ onehot) since engine APs
            # cannot start at a nonzero partition.
            onehot = const.tile([1, 128], F32)
            nc.vector.memset(onehot[:], 0.0)
            nc.vector.memset(onehot[:, 127:128], 1.0)
            onehot6 = const.tile([1, 128], F32)
            nc.vector.memset(onehot6[:], 0.0)
            nc.vector.memset(onehot6[:, 126:127], 1.0)
            sig0 = const.tile([1, 1], F32)
            nc.scalar.activation(sig0[:], sc_sb[:, 10:11],
                                 mybir.ActivationFunctionType.Sigmoid)
            ctr0 = const.tile([1, 1], F32)
            nc.vector.tensor_tensor(out=ctr0[:], in0=sig0[:], in1=sc_sb[:, 4:5],
                                    op=mybir.AluOpType.add)
            halo_m = []
            for dc in range(3):
                v = const.tile([1, 2], F32, tag=f"hv_{dc}")
                top = ctr0[:] if dc == 1 else sc_sb[:, 3 + dc:4 + dc]
                nc.vector.tensor_copy(out=v[:, 0:1], in_=top)
                nc.vector.tensor_copy(out=v[:, 1:2], in_=sc_sb[:, 6 + dc:7 + dc])
                v6 = const.tile([1, 2], F32, tag=f"hv6_{dc}")
                nc.vector.tensor_copy(out=v6[:, 0:1], in_=sc_sb[:, 6 + dc:7 + dc])
                nc.vector.memset(v6[:, 1:2], 0.0)
                ph = psw.tile([2, 128], F32, tag="pw")
                nc.tensor.matmul(ph[:], v[:], onehot[:], start=True, stop=False)
                nc.tensor.matmul(ph[:], v6[:], onehot6[:], start=False, stop=True)
                hb = const.tile([2, 128], BF16, tag=f"hb_{dc}")
                nc.vector.tensor_copy(out=hb[:], in_=ph[:])
                halo_m.append(hb)

            wi = wipool.tile([128, n_win, fin], BF16)      # W_i, fout-major
            wt = wtpool.tile([128, n_k, FSH], BF16)        # W_i^T, fin-major

            for _ in range(repeat):
                # ---- phase T: weight transform --------------------------------
                for w in range(n_win):
                    wrow = wrowp.tile([128, fin + 2], BF16, tag="wrow")
                    nc.gpsimd.dma_start(out=wrow[:],
                                        in_=wh[128 * w:128 * w + 128, :])
                    hrow = wrowp.tile([2, fin + 2], BF16, tag="hrow")
                    nc.gpsimd.dma_start(out=hrow[:],
                                        in_=wh[128 * w + 128:128 * w + 130, :])
                    for s in range(n_strip):
                        pw = psw.tile([128, 512], F32, tag="pw")
                        for dc in range(3):
                            nc.tensor.matmul(
                                pw[:], b_bf[dc][:],
                                wrow[:, 512 * s + dc:512 * s + dc + 512],
                                start=(dc == 0), stop=False)
                        for dc in range(3):
                            nc.tensor.matmul(
                                pw[:], halo_m[dc][:],
                                hrow[:, 512 * s + dc:512 * s + dc + 512],
                                start=False, stop=(dc == 2))
                        # PSUM -> SBUF with bias add, cast to bf16
                        nc.vector.tensor_scalar(
                            wi[:, w, 512 * s:512 * s + 512], pw[:],
                            scv[:, 9:10], None, mybir.AluOpType.add)

                # ---- phase T2: transpose W_i -> W_i^T -------------------------
                for ko in range(n_k):
                    pt = pst.tile([128, 512], BF16, tag="pt")
                    for mo in range(n_win):
                        nc.tensor.transpose(
                            pt[:, 128 * mo:128 * mo + 128],
                            wi[:, mo, 128 * ko:128 * ko + 128],
                            ident[:])
                    if ko % 2 == 0:
                        nc.vector.tensor_copy(out=wt[:, ko, :], in_=pt[:])
                    else:
                        nc.scalar.copy(out=wt[:, ko, :], in_=pt[:])

                # ---- phase M: main matmul -------------------------------------
                for t in range(n_tblk):
                    xb = xbp.tile([128, fin], BF16, tag="xb")
                    nc.gpsimd.dma_start(out=xb[:],
                                        in_=inp[128 * t:128 * t + 128, :])
                    xt = xtp.tile([128, n_k, 128], BF16, tag="xt")
                    for ko in range(n_k // 4):
                        px = psx.tile([128, 512], BF16, tag="px")
                        for ki in range(4):
                            k = 4 * ko + ki
                            nc.tensor.transpose(
                                px[:, 128 * ki:128 * ki + 128],
                                xb[:, 128 * k:128 * k + 128],
                                ident[:])
                        dst = xt[:, 4 * ko:4 * ko + 4, :]
                        if ko % 2 == 0:
                            nc.vector.tensor_copy(out=dst, in_=px[:])
                        else:
                            nc.scalar.copy(out=dst, in_=px[:])
                    po = pso.tile([128, FSH], F32, tag="po")
                    for k in range(n_k):
                        nc.tensor.matmul(po[:], xt[:, k, :], wt[:, k, :],
                                         start=(k == 0), stop=(k == n_k - 1))
                    ob = osbp.tile([128, FSH], F32, tag="ob")
                    nc.scalar.copy(out=ob[:], in_=po[:])
                    nc.sync.dma_start(out=out[128 * t:128 * t + 128, :],
                                      in_=ob[:])

    nc.compile()
    return nc


def shard_inputs(inp, W, conv_w, conv_b, sk_wt, fin=FIN):
    """Build the 8 per-core input maps (W fout-shard with conv halo)."""
    tok = inp.size // fin
    inp2 = np.ascontiguousarray(inp.reshape(tok, fin), dtype=np.float32)
    W = np.asarray(W, dtype=np.float32)
    hsz = W.shape[0] // NUM_HEADS  # rows per head
    in_maps = []
    for c in range(NCORES):
        gr0 = c * FSH
        h = (gr0 // hsz) % NUM_HEADS
        whal = np.zeros((FSH + 2, fin + 2), dtype=np.float32)
        lo = max(gr0 - 1, h * hsz)
        hi = min(gr0 + FSH + 1, (h + 1) * hsz)
        whal[lo - (gr0 - 1):hi - (gr0 - 1), 1:fin + 1] = W[lo:hi, :fin]
        scal = np.zeros((1, 11), dtype=np.float32)
        scal[0, :9] = np.asarray(conv_w, dtype=np.float32)[h].reshape(9)
        scal[0, 9] = np.float32(np.asarray(conv_b)[h])
        scal[0, 10] = np.float32(np.asarray(sk_wt)[h].reshape(()))
        in_maps.append({"inp": inp2, "wh": whal, "sc": scal})
    return in_maps


_PROGRAM_CACHE = {}


def _get_program(tok, fin, repeat=1):
    key = (tok, fin, repeat)
    if key not in _PROGRAM_CACHE:
        _PROGRAM_CACHE[key] = build_program(tok, fin, repeat)
    return _PROGRAM_CACHE[key]


def kernel(inp, W, conv_w, conv_b, sk_wt):
    nc = _get_program(TOK, FIN)
    in_maps = shard_inputs(inp, W, conv_w, conv_b, sk_wt)
    res = run_bass_kernel_spmd(nc, in_maps, list(range(NCORES)))
    shards = [res.results[c]["o"].reshape(2, TOK // 2, FSH)
              for c in range(NCORES)]
    return np.ascontiguousarray(
        np.concatenate(shards, axis=-1).astype(np.float32))
